# revision 1
# baseline (speedup 1.0000x reference)
"""Trainium2 Bass kernel v2 for nn_CategoricalDecoder (topk_masking).

Phase A (bin-sharded): tail-feature logits + local top-16 (3-term f32r
split matmuls). AllToAll flips to batch sharding. Phase B: merge, gather
winning z rows, exact fp32-class recompute of num/den on the 512 selected
columns, logsumexp.
"""

import numpy as np
from contextlib import ExitStack

import bass_rust as _br
import concourse.bass as bass
import concourse.bacc as bacc
import concourse.tile as tile
from concourse import mybir
from concourse.bass_utils import run_bass_kernel_spmd
from concourse.hw_specs import get_activation_tables

F32 = mybir.dt.float32
F32R = mybir.dt.float32r
U16 = mybir.dt.uint16
I16 = mybir.dt.int16
AF = mybir.ActivationFunctionType
ALU = mybir.AluOpType
AX = mybir.AxisListType

B, N, Lz, H, D, C = 256, 8192, 64, 256, 32, 16
DC = D * C
P = 8
NL = N // P
BL = B // P
K = 16
NEG = -1.0e30

# pk64 column offsets
O_ZTSH, O_ZTSL, O_W1H, O_W1L, O_OHT, O_B2T, O_G4 = (
    0, 1024, 2048, 2304, 2560, 2816, 2817)
PK64_COLS = 2821
# pk128 column offsets
O_W2H, O_W2L, O_B1, O_B2, O_OHB, O_GSEL, O_COEF, O_ONES, O_CO, O_IOTA, O_NCBT = (
    0, 1024, 2048, 2050, 2054, 2182, 2310, 2438, 2440, 2441, 2569)
PK128_COLS = 2570


class _Bacc(bacc.Bacc):
    """Bacc that pins every activation to the one table holding
    {Relu, Exp, Ln, Copy}, avoiding per-switch ACT_TABLE_LOADs."""

    def insert_act_table_loads(self):
        has_act = any(isinstance(i, mybir.InstActivation)
                      for b in self.main_func.blocks for i in b.instructions)
        if not has_act:
            return
        tables = []
        for name, funcs in get_activation_tables(self.m.arch).items():
            keep = funcs if name == "natural_log_exp_and_others" else set()
            tables.append((name, keep))
        _br.insert_act_table_loads(self, tables)


def _build_nc():
    nc = _Bacc("TRN2", target_bir_lowering=False, num_devices=P)

    dp = nc.declare_dram_parameter
    pk64 = dp("pk64", [Lz, PK64_COLS], F32R, isOutput=False)
    pk128 = dp("pk128", [128, PK128_COLS], F32R, isOutput=False)
    ztf = dp("ztf", [Lz, N], F32, isOutput=False)
    outp = dp("out", [BL], F32, isOutput=True)

    with tile.TileContext(nc) as tc, ExitStack() as ctx:
        const = ctx.enter_context(tc.tile_pool(name="const", bufs=1))
        dram = ctx.enter_context(tc.tile_pool(name="dram", bufs=1, space="DRAM"))

        k64 = const.tile([Lz, PK64_COLS], F32R, name="k64")
        nc.sync.dma_start(k64[:], pk64[:])
        k128 = const.tile([128, PK128_COLS], F32R, name="k128")
        nc.sync.dma_start(k128[:], pk128[:])
        ztf_sb = const.tile([Lz, N], F32, name="ztf_sb")
        nc.sync.dma_start(ztf_sb[:], ztf[:])

        def c64(off, w, p=Lz, dt=None):
            ap = k64[0:p, off:off + w]
            return ap.bitcast(dt) if dt else ap

        def c128(off, w, p=128, dt=None):
            ap = k128[0:p, off:off + w]
            return ap.bitcast(dt) if dt else ap

        xin = dram.tile([B, 16], F32)
        xout = dram.tile([B, 16], F32)

        # early dummy ap_gather: forces the gpsimd gather library load to
        # overlap the parameter DMAs instead of stalling phase B.
        with ExitStack() as ctx0:
            pre = ctx0.enter_context(tc.tile_pool(name="pre", bufs=1))
            zidx = pre.tile([16, 1], I16, name="zidx")
            nc.vector.memset(zidx[:], 0)
            junkg = pre.tile([16, 16], F32, name="junkg")
            nc.gpsimd.ap_gather(junkg[:], k64[0:16, 0:64].bitcast(F32), zidx[:],
                                channels=16, num_elems=64, d=1, num_idxs=16)

        # ================= phase A =================
        with ExitStack() as ctxA:
            pa = ctxA.enter_context(tc.tile_pool(name="pa", bufs=3, space="PSUM"))
            sp = ctxA.enter_context(tc.tile_pool(name="sp", bufs=1, space="PSUM"))
            act = ctxA.enter_context(tc.tile_pool(name="actA", bufs=1))
            scratch = ctxA.enter_context(tc.tile_pool(name="scrA", bufs=1))

            # hT = relu(W1.T @ zT + b1), 3-term f32r
            hs = []
            for m in range(2):
                ph = pa.tile([128, NL], F32, tag="mm")
                for f in range(2):
                    sl = slice(f * 512, (f + 1) * 512)
                    w1h = c64(O_W1H + m * 128, 128)
                    w1l = c64(O_W1L + m * 128, 128)
                    zh = c64(O_ZTSH + f * 512, 512)
                    zl = c64(O_ZTSL + f * 512, 512)
                    nc.tensor.matmul(ph[:, sl], w1h, zh, start=True, stop=False)
                    nc.tensor.matmul(ph[:, sl], w1h, zl, start=False, stop=False)
                    nc.tensor.matmul(ph[:, sl], w1l, zh, start=False, stop=True)
                b1 = c128(O_B1 + m, 1, dt=F32)
                hh = act.tile([128, NL], F32R, name=f"hh{m}")
                nc.scalar.activation(hh[:], ph[:], AF.Relu, bias=b1)
                hf = act.tile([128, NL], F32, name=f"hf{m}")
                nc.scalar.activation(hf[:], ph[:], AF.Relu, bias=b1)
                hl = act.tile([128, NL], F32R, name=f"hl{m}")
                nc.vector.tensor_sub(hl[:], hf[:], hh[:].bitcast(F32))
                hs.append((hh, hl))

            # tail logits (dc 448..512): [64, NL] (b2 folded out on host)
            pl3 = pa.tile([128, NL], F32, tag="mm")
            for f in range(2):
                sl = slice(f * 512, (f + 1) * 512)
                for kk in range(2):
                    w2h = c128(O_W2H + kk * DC + 448, 64)
                    w2l = c128(O_W2L + kk * DC + 448, 64)
                    hh, hl = hs[kk]
                    nc.tensor.matmul(pl3[0:64, sl], w2h, hh[:, sl],
                                     start=(kk == 0), stop=False)
                    nc.tensor.matmul(pl3[0:64, sl], w2h, hl[:, sl],
                                     start=False, stop=False)
                    nc.tensor.matmul(pl3[0:64, sl], w2l, hh[:, sl],
                                     start=False, stop=(kk == 1))
            b2t = c64(O_B2T, 1, dt=F32)
            e3r = act.tile([Lz, NL], F32R, name="e3r")
            nc.scalar.activation(e3r[:], pl3[0:64, :], AF.Exp, bias=b2t)
            l3h = act.tile([Lz, NL], F32R, name="l3h")
            nc.scalar.copy(l3h[:], pl3[0:64, :])
            l3l = act.tile([Lz, NL], F32R, name="l3l")
            nc.vector.tensor_sub(l3l[:], pl3[0:64, :], l3h[:].bitcast(F32))

            # log-sumexp of the 4 tail feature groups
            pse4 = sp.tile([4, NL], F32, tag="se")
            for f in range(2):
                sl = slice(f * 512, (f + 1) * 512)
                nc.tensor.matmul(pse4[:, sl], c64(O_G4, 4), e3r[:, sl],
                                 start=True, stop=True)
            l4h = act.tile([4, NL], F32R, name="l4h")
            nc.scalar.activation(l4h[:], pse4[:], AF.Ln)
            l4f = act.tile([4, NL], F32, name="l4f")
            nc.scalar.activation(l4f[:], pse4[:], AF.Ln)
            l4l = act.tile([4, NL], F32R, name="l4l")
            nc.vector.tensor_sub(l4l[:], l4f[:], l4h[:].bitcast(F32))

            # tail scores st[bt] [128, NL] = oht.T @ logits3 - sum(l4)
            for bt in range(2):
                pst = pa.tile([128, NL], F32, tag="mm")
                for f in range(2):
                    sl = slice(f * 512, (f + 1) * 512)
                    oht = c64(O_OHT + bt * 128, 128)
                    nc.tensor.matmul(pst[:, sl], oht, l3h[:, sl],
                                     start=True, stop=False)
                    nc.tensor.matmul(pst[:, sl], oht, l3l[:, sl],
                                     start=False, stop=False)
                    nc.tensor.matmul(pst[:, sl], c128(O_COEF, 128, p=4),
                                     l4h[:, sl], start=False, stop=False)
                    nc.tensor.matmul(pst[:, sl], c128(O_COEF, 128, p=4),
                                     l4l[:, sl], start=False, stop=True)

                # local top-8 + global ids, straight from PSUM
                x_sb = act.tile([128, 16], F32, name=f"x{bt}")
                nc.vector.max(x_sb[:, 0:8], pst[:])
                pu = act.tile([128, 8], U16, name=f"pu{bt}")
                nc.vector.max_index(pu[:], x_sb[:, 0:8], pst[:])
                nc.vector.tensor_copy(x_sb[:, 8:16], pu[:])
                nc.vector.tensor_scalar_add(x_sb[:, 8:16], x_sb[:, 8:16],
                                            c128(O_CO, 1, dt=F32))
                nc.sync.dma_start(xin[bt * 128:(bt + 1) * 128, :], x_sb[:])

        nc.gpsimd.collective_compute(
            "AllToAll", ALU.bypass, replica_groups=[list(range(P))],
            ins=[xin[:].opt()], outs=[xout[:].opt()],
        )

        # ================= phase B =================
        with ExitStack() as ctxB:
            pb = ctxB.enter_context(tc.tile_pool(name="pb", bufs=4, space="PSUM"))
            spb = ctxB.enter_context(tc.tile_pool(name="spb", bufs=1, space="PSUM"))
            act = ctxB.enter_context(tc.tile_pool(name="actB", bufs=1))
            scratch = ctxB.enter_context(tc.tile_pool(name="scrB", bufs=1))

            y = act.tile([BL, P, 16], F32, name="y")
            nc.sync.dma_start(y[:], xout[:].rearrange("(s p) f -> p s f", s=P))
            cands = act.tile([BL, P * 8], F32, name="cands")
            nc.vector.tensor_copy(
                cands[:].rearrange("p (a b) -> p a b", a=P), y[:, :, 0:8])
            idxc = act.tile([BL, P * 8], F32, name="idxc")
            nc.vector.tensor_copy(
                idxc[:].rearrange("p (a b) -> p a b", a=P), y[:, :, 8:16])

            wv = act.tile([BL, 16], F32, name="wv")
            nc.vector.max(wv[:, 0:8], cands[:])
            cm = act.tile([BL, P * 8], F32, name="cm")
            nc.vector.match_replace(cm[:], wv[:, 0:8], cands[:], NEG)
            nc.vector.max(wv[:, 8:16], cm[:])
            pw = act.tile([BL, 16], U16, name="pw")
            nc.vector.max_index(pw[:, 0:8], wv[:, 0:8], cands[:])
            nc.vector.max_index(pw[:, 8:16], wv[:, 8:16], cm[:])
            posf = act.tile([BL, 16], F32, name="posf")
            nc.vector.tensor_copy(posf[:], pw[:])

            widp = act.tile([32, 32], F32, name="widp")
            for j in range(16):
                junk = scratch.tile([BL, P * 8], F32, tag="junk")
                nc.vector.scalar_tensor_tensor(
                    junk[:], c128(O_IOTA, P * 8, p=BL, dt=F32), posf[:, j:j + 1],
                    idxc[:], op0=ALU.is_equal, op1=ALU.mult,
                    accum_out=widp[0:BL, j:j + 1])
            tp = act.tile([32, 32], F32, name="tp")
            nc.vector.transpose(tp[:], widp[:])
            idx64 = act.tile([Lz, 32], I16, name="idx64")
            nc.vector.tensor_copy(idx64[0:16, :], tp[0:16, :])
            for g in range(1, 4):
                nc.sync.dma_start(idx64[16 * g:16 * (g + 1), :], idx64[0:16, :])

            ztop = act.tile([Lz, 512], F32, name="ztop")
            nc.gpsimd.ap_gather(ztop[:], ztf_sb[:], idx64[:],
                                channels=Lz, num_elems=N, d=1, num_idxs=512)
            zh = act.tile([Lz, 512], F32R, name="zh")
            nc.vector.tensor_copy(zh[:], ztop[:])
            zl = act.tile([Lz, 512], F32R, name="zl")
            nc.vector.tensor_sub(zl[:], ztop[:], zh[:].bitcast(F32))

            h2s = []
            for m in range(2):
                ph2 = pb.tile([128, 512], F32, tag="mmb")
                w1h = c64(O_W1H + m * 128, 128)
                w1l = c64(O_W1L + m * 128, 128)
                nc.tensor.matmul(ph2[:], w1h, zh[:], start=True, stop=False)
                nc.tensor.matmul(ph2[:], w1h, zl[:], start=False, stop=False)
                nc.tensor.matmul(ph2[:], w1l, zh[:], start=False, stop=True)
                b1 = c128(O_B1 + m, 1, dt=F32)
                hh = act.tile([128, 512], F32R, name=f"hh2{m}")
                nc.scalar.activation(hh[:], ph2[:], AF.Relu, bias=b1)
                hf = act.tile([128, 512], F32, name=f"hf2{m}")
                nc.scalar.activation(hf[:], ph2[:], AF.Relu, bias=b1)
                hl = act.tile([128, 512], F32R, name=f"hl2{m}")
                nc.vector.tensor_sub(hl[:], hf[:], hh[:].bitcast(F32))
                h2s.append((hh, hl))

            pse2 = spb.tile([32, 512], F32, tag="seb")
            lin2s = []
            for t in range(4):
                pl2 = pb.tile([128, 512], F32, tag="mmb")
                for kk in range(2):
                    w2h = c128(O_W2H + kk * DC + t * 128, 128)
                    w2l = c128(O_W2L + kk * DC + t * 128, 128)
                    hh, hl = h2s[kk]
                    nc.tensor.matmul(pl2[:], w2h, hh[:], start=(kk == 0), stop=False)
                    nc.tensor.matmul(pl2[:], w2h, hl[:], start=False, stop=False)
                    nc.tensor.matmul(pl2[:], w2l, hh[:], start=False, stop=(kk == 1))
                b2 = c128(O_B2 + t, 1, dt=F32)
                e2r = act.tile([128, 512], F32R, name=f"e2r{t}")
                nc.scalar.activation(e2r[:], pl2[:], AF.Exp, bias=b2)
                lh = act.tile([128, 512], F32R, name=f"l2h{t}")
                nc.scalar.copy(lh[:], pl2[:])
                ll = act.tile([128, 512], F32R, name=f"l2l{t}")
                nc.vector.tensor_sub(ll[:], pl2[:], lh[:].bitcast(F32))
                lin2s.append((lh, ll))
                nc.tensor.matmul(pse2[:], c128(O_GSEL + t * 32, 32), e2r[:],
                                 start=(t == 0), stop=(t == 3))
            lgh = act.tile([32, 512], F32R, name="lgh")
            nc.scalar.activation(lgh[:], pse2[:], AF.Ln)
            lgf = act.tile([32, 512], F32, name="lgf")
            nc.scalar.activation(lgf[:], pse2[:], AF.Ln)
            lgl = act.tile([32, 512], F32R, name="lgl")
            nc.vector.tensor_sub(lgl[:], lgf[:], lgh[:].bitcast(F32))

            pnum = pb.tile([BL, 512], F32, tag="mmb")
            for t in range(4):
                lh, ll = lin2s[t]
                ohb = c128(O_OHB + t * BL, BL)
                nc.tensor.matmul(pnum[:], ohb, lh[:], start=(t == 0), stop=False)
                nc.tensor.matmul(pnum[:], ohb, ll[:], start=False, stop=False)
            nc.tensor.matmul(pnum[:], c128(O_COEF, BL, p=32), lgh[:],
                             start=False, stop=False)
            nc.tensor.matmul(pnum[:], c128(O_COEF, BL, p=32), lgl[:],
                             start=False, stop=True)
            numfull = act.tile([BL, 512], F32, name="numfull")
            nc.vector.tensor_copy(numfull[:], pnum[:])
            dscr = dram.tile([BL, 512], F32)
            nc.sync.dma_start(dscr[:], numfull[:])
            numd = act.tile([BL, 16], F32, name="numd")
            diag = bass.AP(tensor=dscr[:].tensor, offset=0,
                           ap=[[512 + 16, BL], [1, 16]])
            nc.sync.dma_start(numd[:], diag)

            # den = (numd + (-cbt)) - wv   (cbt: host-side tail-bias fold)
            den = act.tile([BL, 16], F32, name="den")
            nc.vector.scalar_tensor_tensor(
                den[:], numd[:], c128(O_NCBT, 1, p=BL, dt=F32), wv[:],
                op0=ALU.add, op1=ALU.subtract)
            ng = act.tile([BL, 2], F32, name="ng")
            nc.vector.tensor_reduce(ng[:, 0:1], numd[:], axis=AX.X, op=ALU.max,
                                    negate=True)
            nc.vector.tensor_reduce(ng[:, 1:2], den[:], axis=AX.X, op=ALU.max,
                                    negate=True)
            s2 = act.tile([BL, 2], F32, name="s2")
            en = scratch.tile([BL, 16], F32, tag="ex")
            nc.scalar.activation(en[:], numd[:], AF.Exp, bias=ng[:, 0:1],
                                 accum_out=s2[:, 0:1])
            ed = scratch.tile([BL, 16], F32, tag="ex")
            nc.scalar.activation(ed[:], den[:], AF.Exp, bias=ng[:, 1:2],
                                 accum_out=s2[:, 1:2])
            lg = act.tile([BL, 2], F32, name="lg")
            nc.scalar.activation(lg[:], s2[:], AF.Ln)
            t1 = act.tile([BL, 1], F32, name="t1")
            nc.vector.tensor_sub(t1[:], lg[:, 0:1], lg[:, 1:2])
            t2 = act.tile([BL, 1], F32, name="t2")
            nc.vector.tensor_sub(t2[:], ng[:, 1:2], ng[:, 0:1])
            t3 = act.tile([BL, 1], F32, name="t3")
            nc.vector.tensor_add(t3[:], t1[:], t2[:])
            nc.sync.dma_start(outp[:], t3[:, 0])

    nc.compile()
    return nc


def _trunc_split(a):
    a = np.ascontiguousarray(a, np.float32)
    hi = (a.view(np.uint32) & np.uint32(0xFFFFF000)).view(np.float32)
    lo = a - hi
    return hi, lo


def _host_prep(x, z, W1, b1, W2, b2):
    oh = np.zeros((B, DC), np.float32)
    oh[np.arange(B)[:, None], np.arange(D)[None, :] * C + x] = 1.0
    ohT = np.ascontiguousarray(oh.T)
    w2s = np.ascontiguousarray(
        W2.reshape(2, 128, DC).transpose(1, 0, 2).reshape(128, 2 * DC))
    w2h, w2l = _trunc_split(w2s)
    w1h, w1l = _trunc_split(W1)
    cbt = oh[:, 448:512] @ b2[448:512]          # (256,)

    k64c = np.zeros((Lz, PK64_COLS), np.float32)
    k64c[:, O_W1H:O_W1H + H] = w1h
    k64c[:, O_W1L:O_W1L + H] = w1l
    k64c[:, O_OHT:O_OHT + B] = ohT[448:512, :]
    k64c[:, O_B2T] = b2[448:512]
    g4 = np.zeros((Lz, 4), np.float32)
    g4[np.arange(Lz), np.arange(Lz) // 16] = 1.0
    k64c[:, O_G4:O_G4 + 4] = g4

    k128c = np.zeros((128, PK128_COLS), np.float32)
    k128c[:, O_W2H:O_W2H + 2 * DC] = w2h
    k128c[:, O_W2L:O_W2L + 2 * DC] = w2l
    k128c[:, O_B1:O_B1 + 2] = b1.reshape(2, 128).T
    k128c[:, O_B2:O_B2 + 4] = b2.reshape(4, 128).T
    p_idx = np.arange(128)
    for t in range(4):
        k128c[p_idx, O_GSEL + t * 32 + t * 8 + p_idx // 16] = 1.0
    k128c[0:32, O_COEF:O_COEF + 128] = -1.0
    k128c[:, O_ONES] = 1.0
    k128c[0:BL, O_IOTA:O_IOTA + 128] = np.arange(128, dtype=np.float32)[None, :]

    ztfull = np.ascontiguousarray(z.T)
    in_maps = []
    for c in range(P):
        kc64 = k64c.copy()
        zsh, zsl = _trunc_split(z[c * NL:(c + 1) * NL, :].T)
        kc64[:, O_ZTSH:O_ZTSH + NL] = zsh
        kc64[:, O_ZTSL:O_ZTSL + NL] = zsl
        kc128 = k128c.copy()
        kc128[:, O_CO] = c * NL
        for t in range(4):
            kc128[:, O_OHB + t * BL:O_OHB + (t + 1) * BL] = \
                ohT[t * 128:(t + 1) * 128, c * BL:(c + 1) * BL]
        kc128[0:BL, O_NCBT] = -cbt[c * BL:(c + 1) * BL]
        in_maps.append(dict(pk64=kc64, pk128=kc128, ztf=ztfull))
    return in_maps


_NC_CACHE = {}


def kernel(x, log_w, z, k, W1, b1, W2, b2, _trace=False, _trace_kwargs=None):
    assert int(k) == K
    in_maps = _host_prep(np.asarray(x, np.int32), np.asarray(z, np.float32),
                         np.asarray(W1, np.float32), np.asarray(b1, np.float32),
                         np.asarray(W2, np.float32), np.asarray(b2, np.float32))
    if "nc" not in _NC_CACHE:
        _NC_CACHE["nc"] = _build_nc()
    nc = _NC_CACHE["nc"]
    res = run_bass_kernel_spmd(
        nc, in_maps, list(range(P)), trace=_trace, **(_trace_kwargs or {}))
    if _trace:
        _NC_CACHE["last_result"] = res
    return np.concatenate([np.asarray(res.results[c]["out"], np.float32)
                           for c in range(P)])



# revision 4
# speedup vs baseline: 1.1241x; 1.1241x over previous
"""Trainium2 Bass kernel v3 for nn_CategoricalDecoder (topk_masking).

Single-pass bin-sharded design: each core scores its NL=1024 bins,
computing BOTH the 4-feature tail score (3-term f32r precision — it
reaches the output directly) and the 28-feature head score (1-term f32r
— its error cancels to first order between lse(num) and lse(den)).
Per batch row the local top-8 (tail, head) value pairs ride the 16KB
AllToAll; the post-collective phase is just a threshold-masked
logsumexp. No z gather, no second net pass.
"""

import numpy as np
from contextlib import ExitStack

import bass_rust as _br
import concourse.bass as bass
import concourse.bacc as bacc
import concourse.tile as tile
from concourse import mybir
from concourse.bass_utils import run_bass_kernel_spmd
from concourse.hw_specs import get_activation_tables

F32 = mybir.dt.float32
F32R = mybir.dt.float32r
U16 = mybir.dt.uint16
I16 = mybir.dt.int16
AF = mybir.ActivationFunctionType
ALU = mybir.AluOpType
AX = mybir.AxisListType

B, N, Lz, H, D, C = 256, 8192, 64, 256, 32, 16
DC = D * C
P = 8
NL = N // P
BL = B // P
K = 16
NEG = -1.0e30

# pk64 column offsets (partition dim 64)
O_ZTSH, O_ZTSL, O_W1H, O_W1L, O_OHT = 0, 1024, 2048, 2304, 2560
PK64_COLS = 2816
# pk128 column offsets (partition dim 128)
O_W2R, O_W2TH, O_W2TL, O_OHM, O_OHM3, O_GSEL, O_GS3 = (
    0, 1024, 1152, 1280, 2304, 2560, 2656)
O_COEFH, O_COEFT, O_IOTA, O_CSEL, O_B1, O_B2C, O_B2G, O_CBT = (
    2720, 2848, 2976, 3104, 3112, 3114, 3118, 3120)
PK128_COLS = 3121


class _Bacc(bacc.Bacc):
    """Bacc that pins every activation to the one table holding
    {Relu, Exp, Ln, Copy}, avoiding per-switch ACT_TABLE_LOADs."""

    def insert_act_table_loads(self):
        has_act = any(isinstance(i, mybir.InstActivation)
                      for b in self.main_func.blocks for i in b.instructions)
        if not has_act:
            return
        tables = []
        for name, funcs in get_activation_tables(self.m.arch).items():
            keep = funcs if name == "natural_log_exp_and_others" else set()
            tables.append((name, keep))
        _br.insert_act_table_loads(self, tables)


def _build_nc():
    nc = _Bacc("TRN2", target_bir_lowering=False, num_devices=P)

    dp = nc.declare_dram_parameter
    pk64 = dp("pk64", [Lz, PK64_COLS], F32R, isOutput=False)
    pk128 = dp("pk128", [128, PK128_COLS], F32R, isOutput=False)
    outp = dp("out", [BL], F32, isOutput=True)

    with tile.TileContext(nc) as tc, ExitStack() as ctx:
        const = ctx.enter_context(tc.tile_pool(name="const", bufs=1))
        dram = ctx.enter_context(tc.tile_pool(name="dram", bufs=1, space="DRAM"))

        k64 = const.tile([Lz, PK64_COLS], F32R, name="k64")
        nc.sync.dma_start(k64[:], pk64[:])
        k128 = const.tile([128, PK128_COLS], F32R, name="k128")
        # w2 parts land first (needed by the l matmuls), bulky one-hot later
        nc.sync.dma_start(k128[:, 0:O_OHM], pk128[:, 0:O_OHM])
        nc.sync.dma_start(k128[:, O_OHM:], pk128[:, O_OHM:])

        def c64(off, w, p=Lz, dt=None):
            ap = k64[0:p, off:off + w]
            return ap.bitcast(dt) if dt else ap

        def c128(off, w, p=128, dt=None):
            ap = k128[0:p, off:off + w]
            return ap.bitcast(dt) if dt else ap

        xin = dram.tile([B, 16], F32)
        xout = dram.tile([B, 16], F32)

        # early dummy ap_gather: exercises the gpsimd gather library so its
        # load overlaps the parameter DMAs. Fed from memset tiles (no DMA dep).
        with ExitStack() as ctx0:
            pre = ctx0.enter_context(tc.tile_pool(name="pre", bufs=1))
            zidx = pre.tile([16, 4], I16, name="zidx")
            nc.vector.memset(zidx[:], 0)
            jtab = pre.tile([16, 64], F32, name="jtab")
            nc.vector.memset(jtab[:], 0)
            junkg = pre.tile([16, 64], F32, name="junkg")
            nc.gpsimd.ap_gather(junkg[:], jtab[:], zidx[:],
                                channels=16, num_elems=64, d=1, num_idxs=64)

        # PE warmup: junk matmuls during the DMA wait keep HAM from
        # throttling the array to 1.2 GHz when real work starts.
        with ExitStack() as ctxW:
            wsb = ctxW.enter_context(tc.tile_pool(name="wsb", bufs=1))
            wps = ctxW.enter_context(tc.tile_pool(name="wps", bufs=2, space="PSUM"))
            wj = wsb.tile([128, 512], F32, name="wj")
            nc.vector.memset(wj[:], 0)
            for g in range(4):
                wp = wps.tile([128, 512], F32, tag="wp")
                for i in range(6):
                    nc.tensor.matmul(wp[:], wj[:, 0:128].bitcast(F32R),
                                     wj[:].bitcast(F32R),
                                     start=(i == 0), stop=(i == 5))

        act = ctx.enter_context(tc.tile_pool(name="act", bufs=1))
        scr = ctx.enter_context(tc.tile_pool(name="scr", bufs=3))

        # ---------------- phase A: h = relu(W1.T @ zT + b1) ----------------
        hh = [act.tile([128, NL], F32R, name=f"hh{m}") for m in range(2)]
        hl = [act.tile([128, NL], F32R, name=f"hl{m}") for m in range(2)]
        lh = [act.tile([128, NL], F32R, name=f"lh{t}") for t in range(3)]
        lh3h = act.tile([Lz, NL], F32R, name="lh3h")
        l3h = act.tile([Lz, NL], F32R, name="l3h")
        l3l = act.tile([Lz, NL], F32R, name="l3l")
        lgh = act.tile([32, NL], F32R, name="lgh")
        lgf = act.tile([32, NL], F32, name="lgf")
        lgl = act.tile([32, NL], F32R, name="lgl")
        den_sb = [act.tile([128, NL], F32, name=f"den{bt}") for bt in range(2)]

        with ExitStack() as ctxA:
            php = ctxA.enter_context(tc.tile_pool(name="php", bufs=2, space="PSUM"))
            plp = ctxA.enter_context(tc.tile_pool(name="plp", bufs=3, space="PSUM"))
            psep = ctxA.enter_context(tc.tile_pool(name="psep", bufs=2, space="PSUM"))

            for m in range(2):
                w1h = c64(O_W1H + m * 128, 128)
                w1l = c64(O_W1L + m * 128, 128)
                b1 = c128(O_B1 + m, 1, dt=F32)
                for f in range(2):
                    sl = slice(f * 512, (f + 1) * 512)
                    zh = c64(O_ZTSH + f * 512, 512)
                    zl = c64(O_ZTSL + f * 512, 512)
                    ph = php.tile([128, 512], F32, tag="ph")
                    nc.tensor.matmul(ph[:], w1h, zh, start=True, stop=False)
                    nc.tensor.matmul(ph[:], w1h, zl, start=False, stop=False)
                    nc.tensor.matmul(ph[:], w1l, zh, start=False, stop=True)
                    nc.scalar.activation(hh[m][:, sl], ph[:], AF.Relu, bias=b1)
                    nc.vector.scalar_tensor_tensor(
                        hl[m][:, sl], ph[:], 0.0, hh[m][:, sl].bitcast(F32),
                        op0=ALU.max, op1=ALU.subtract)

            # ---- logits, exp, per-feature group sums ----
            pse = [psep.tile([32, 512], F32, tag="pse", name=f"pse{f}")
                   for f in range(2)]
            for f in range(2):
                sl = slice(f * 512, (f + 1) * 512)
                for t in range(3):
                    pl = plp.tile([128, 512], F32, tag="pl")
                    for kk in range(2):
                        w2 = c128(O_W2R + kk * 512 + t * 128, 128)
                        nc.tensor.matmul(pl[:], w2, hh[kk][:, sl],
                                         start=(kk == 0), stop=(kk == 1))
                    e = scr.tile([128, 512], F32R, tag="e")
                    nc.scalar.activation(e[:], pl[:], AF.Exp,
                                         bias=c128(O_B2C + t, 1, dt=F32))
                    nc.tensor.matmul(pse[f][:], c128(O_GSEL + t * 32, 32), e[:],
                                     start=(t == 0), stop=False)
                    if t == 1:
                        nc.vector.tensor_copy(lh[t][:, sl], pl[:])
                    else:
                        nc.scalar.copy(lh[t][:, sl], pl[:])
                # t3 head half (dc 384..447), 1-term
                pl3h = plp.tile([Lz, 512], F32, tag="pl")
                for kk in range(2):
                    w2 = c128(O_W2R + kk * 512 + 384, 64)
                    nc.tensor.matmul(pl3h[:], w2, hh[kk][:, sl],
                                     start=(kk == 0), stop=(kk == 1))
                e3h = scr.tile([Lz, 512], F32R, tag="e")
                nc.scalar.activation(e3h[:], pl3h[:], AF.Exp,
                                     bias=c128(O_B2G, 1, p=Lz, dt=F32))
                nc.tensor.matmul(pse[f][:], c128(O_GS3, 32, p=Lz), e3h[:],
                                 start=False, stop=False)
                nc.scalar.copy(lh3h[:, sl], pl3h[:])
                # t3 tail half (dc 448..511), 3-term
                pl3t = plp.tile([Lz, 512], F32, tag="pl")
                for kk in range(2):
                    w2h = c128(O_W2TH + kk * 64, 64)
                    w2l = c128(O_W2TL + kk * 64, 64)
                    nc.tensor.matmul(pl3t[:], w2h, hh[kk][:, sl],
                                     start=(kk == 0), stop=False)
                    nc.tensor.matmul(pl3t[:], w2h, hl[kk][:, sl],
                                     start=False, stop=False)
                    nc.tensor.matmul(pl3t[:], w2l, hh[kk][:, sl],
                                     start=False, stop=(kk == 1))
                e3t = scr.tile([Lz, 512], F32R, tag="e")
                nc.scalar.activation(e3t[:], pl3t[:], AF.Exp,
                                     bias=c128(O_B2G + 1, 1, p=Lz, dt=F32))
                nc.tensor.matmul(pse[f][:], c128(O_GS3 + 32, 32, p=Lz), e3t[:],
                                 start=False, stop=True)
                nc.scalar.copy(l3h[:, sl], pl3t[:])
                nc.vector.tensor_sub(l3l[:, sl], pl3t[:],
                                     l3h[:, sl].bitcast(F32))
                # per-feature log-sum-exp, hi/lo
                nc.scalar.activation(lgh[:, sl], pse[f][:], AF.Ln)
                nc.scalar.activation(lgf[:, sl], pse[f][:], AF.Ln)
                nc.vector.tensor_sub(lgl[:, sl], lgf[:, sl],
                                     lgh[:, sl].bitcast(F32))

        # ---------------- scores + local top-8 + payload ----------------
        with ExitStack() as ctxS:
            pnp = ctxS.enter_context(tc.tile_pool(name="pnp", bufs=2, space="PSUM"))
            ptp = ctxS.enter_context(tc.tile_pool(name="ptp", bufs=2, space="PSUM"))
            for bt in range(2):
                pnum = pnp.tile([128, NL], F32, tag="pnum")
                for f in range(2):
                    sl = slice(f * 512, (f + 1) * 512)
                    for t in range(3):
                        ohm = c128(O_OHM + t * 256 + bt * 128, 128)
                        nc.tensor.matmul(pnum[:, sl], ohm, lh[t][:, sl],
                                         start=(t == 0), stop=False)
                    nc.tensor.matmul(pnum[:, sl],
                                     c128(O_OHM3 + bt * 128, 128, p=Lz),
                                     lh3h[:, sl], start=False, stop=False)
                    nc.tensor.matmul(pnum[:, sl], c128(O_COEFH, 128, p=32),
                                     lgh[:, sl], start=False, stop=True)
                # head scores to SBUF (gather table)
                nc.scalar.copy(den_sb[bt][:, 0:512], pnum[:, 0:512])
                nc.vector.tensor_copy(den_sb[bt][:, 512:1024], pnum[:, 512:1024])

                pst = ptp.tile([128, NL], F32, tag="pst")
                for f in range(2):
                    sl = slice(f * 512, (f + 1) * 512)
                    oht = c64(O_OHT + bt * 128, 128)
                    nc.tensor.matmul(pst[:, sl], oht, l3h[:, sl],
                                     start=True, stop=False)
                    nc.tensor.matmul(pst[:, sl], oht, l3l[:, sl],
                                     start=False, stop=False)
                    nc.tensor.matmul(pst[:, sl], c128(O_COEFT, 128, p=32),
                                     lgh[:, sl], start=False, stop=False)
                    nc.tensor.matmul(pst[:, sl], c128(O_COEFT, 128, p=32),
                                     lgl[:, sl], start=False, stop=True)

                xin_sb = act.tile([128, 16], F32, name=f"xin{bt}")
                nc.vector.max(xin_sb[:, 0:8], pst[:])
                pu = act.tile([128, 8], U16, name=f"pu{bt}")
                nc.vector.max_index(pu[:], xin_sb[:, 0:8], pst[:])
                G = scr.tile([128, 128], F32, tag="G")
                nc.gpsimd.ap_gather(G[:], den_sb[bt][:], pu[:].bitcast(I16),
                                    channels=128, num_elems=NL, d=1,
                                    num_idxs=128)
                for j in range(8):
                    junk = scr.tile([128, 128], F32, tag="jx")
                    nc.vector.scalar_tensor_tensor(
                        junk[:], c128(O_IOTA, 128, dt=F32),
                        c128(O_CSEL + j, 1, dt=F32), G[:],
                        op0=ALU.is_equal, op1=ALU.mult,
                        accum_out=xin_sb[:, 8 + j:9 + j])
                nc.sync.dma_start(xin[bt * 128:(bt + 1) * 128, :], xin_sb[:])

        nc.gpsimd.collective_compute(
            "AllToAll", ALU.bypass, replica_groups=[list(range(P))],
            ins=[xin[:].opt()], outs=[xout[:].opt()],
        )

        # ---------------- merge: threshold top-16, masked logsumexp ----------
        y = act.tile([BL, P, 16], F32, name="y")
        nc.sync.dma_start(y[:], xout[:].rearrange("(s p) f -> p s f", s=P))
        tails = act.tile([BL, P * 8], F32, name="tails")
        nc.vector.tensor_copy(
            tails[:].rearrange("p (a b) -> p a b", a=P), y[:, :, 0:8])
        dens = act.tile([BL, P * 8], F32, name="dens")
        nc.vector.tensor_copy(
            dens[:].rearrange("p (a b) -> p a b", a=P), y[:, :, 8:16])

        wv = act.tile([BL, 16], F32, name="wv")
        nc.vector.max(wv[:, 0:8], tails[:])
        cm = act.tile([BL, P * 8], F32, name="cm")
        nc.vector.match_replace(cm[:], wv[:, 0:8], tails[:], NEG)
        nc.vector.max(wv[:, 8:16], cm[:])

        mask = act.tile([BL, P * 8], F32, name="mask")
        nc.vector.tensor_scalar(mask[:], tails[:], wv[:, 15:16], None,
                                op0=ALU.is_ge)
        num = act.tile([BL, P * 8], F32, name="num")
        nc.vector.tensor_add(num[:], tails[:], dens[:])
        ng = act.tile([BL, 2], F32, name="ng")
        nc.vector.tensor_reduce(ng[:, 0:1], num[:], axis=AX.X, op=ALU.max,
                                negate=True)
        nc.vector.tensor_reduce(ng[:, 1:2], dens[:], axis=AX.X, op=ALU.max,
                                negate=True)
        en = scr.tile([BL, P * 8], F32, tag="ex")
        nc.scalar.activation(en[:], num[:], AF.Exp, bias=ng[:, 0:1])
        ed = scr.tile([BL, P * 8], F32, tag="ex")
        nc.scalar.activation(ed[:], dens[:], AF.Exp, bias=ng[:, 1:2])
        s2 = act.tile([BL, 2], F32, name="s2")
        jm = scr.tile([BL, P * 8], F32, tag="ex")
        nc.vector.scalar_tensor_tensor(jm[:], en[:], 1.0, mask[:],
                                       op0=ALU.mult, op1=ALU.mult,
                                       accum_out=s2[:, 0:1])
        jm2 = scr.tile([BL, P * 8], F32, tag="ex")
        nc.vector.scalar_tensor_tensor(jm2[:], ed[:], 1.0, mask[:],
                                       op0=ALU.mult, op1=ALU.mult,
                                       accum_out=s2[:, 1:2])
        lg = act.tile([BL, 2], F32, name="lg")
        nc.scalar.activation(lg[:], s2[:], AF.Ln)
        t1 = act.tile([BL, 1], F32, name="t1")
        nc.vector.tensor_sub(t1[:], lg[:, 0:1], lg[:, 1:2])
        t2 = act.tile([BL, 1], F32, name="t2")
        nc.vector.tensor_sub(t2[:], ng[:, 1:2], ng[:, 0:1])
        t3 = act.tile([BL, 1], F32, name="t3")
        nc.vector.tensor_add(t3[:], t1[:], t2[:])
        t4 = act.tile([BL, 1], F32, name="t4")
        nc.vector.tensor_add(t4[:], t3[:], c128(O_CBT, 1, p=BL, dt=F32))
        nc.sync.dma_start(outp[:], t4[:, 0])

    nc.compile()
    return nc


def _trunc_split(a):
    a = np.ascontiguousarray(a, np.float32)
    hi = (a.view(np.uint32) & np.uint32(0xFFFFF000)).view(np.float32)
    lo = a - hi
    return hi, lo


def _host_prep(x, z, W1, b1, W2, b2):
    oh = np.zeros((B, DC), np.float32)
    oh[np.arange(B)[:, None], np.arange(D)[None, :] * C + x] = 1.0
    ohT = np.ascontiguousarray(oh.T)
    w2s = np.ascontiguousarray(
        W2.reshape(2, 128, DC).transpose(1, 0, 2).reshape(128, 2 * DC))
    w1h, w1l = _trunc_split(W1)
    cbt = oh[:, 448:512] @ b2[448:512]          # (256,)

    k64c = np.zeros((Lz, PK64_COLS), np.float32)
    k64c[:, O_W1H:O_W1H + H] = w1h
    k64c[:, O_W1L:O_W1L + H] = w1l
    k64c[:, O_OHT:O_OHT + B] = ohT[448:512, :]

    k128c = np.zeros((128, PK128_COLS), np.float32)
    k128c[:, O_W2R:O_W2R + 2 * DC] = w2s
    for kk in range(2):
        th, tl = _trunc_split(w2s[:, kk * DC + 448:kk * DC + 512])
        k128c[:, O_W2TH + kk * 64:O_W2TH + (kk + 1) * 64] = th
        k128c[:, O_W2TL + kk * 64:O_W2TL + (kk + 1) * 64] = tl
    for t in range(3):
        for bt in range(2):
            k128c[:, O_OHM + t * 256 + bt * 128:O_OHM + t * 256 + (bt + 1) * 128] = \
                ohT[t * 128:(t + 1) * 128, bt * 128:(bt + 1) * 128]
    for bt in range(2):
        k128c[0:Lz, O_OHM3 + bt * 128:O_OHM3 + (bt + 1) * 128] = \
            ohT[384:448, bt * 128:(bt + 1) * 128]
    p_idx = np.arange(128)
    for t in range(3):
        k128c[p_idx, O_GSEL + t * 32 + t * 8 + p_idx // 16] = 1.0
    p64 = np.arange(Lz)
    k128c[p64, O_GS3 + 24 + p64 // 16] = 1.0
    k128c[p64, O_GS3 + 32 + 28 + p64 // 16] = 1.0
    k128c[0:28, O_COEFH:O_COEFH + 128] = -1.0
    k128c[28:32, O_COEFT:O_COEFT + 128] = -1.0
    k128c[:, O_IOTA:O_IOTA + 128] = np.arange(128, dtype=np.float32)[None, :]
    k128c[:, O_CSEL:O_CSEL + 8] = (16.0 * np.arange(8)[None, :]
                                   + (p_idx % 16)[:, None])
    k128c[:, O_B1:O_B1 + 2] = b1.reshape(2, 128).T
    k128c[:, O_B2C:O_B2C + 4] = b2.reshape(4, 128).T
    k128c[0:Lz, O_B2G:O_B2G + 2] = b2[384:512].reshape(2, Lz).T

    in_maps = []
    for c in range(P):
        kc64 = k64c.copy()
        zsh, zsl = _trunc_split(z[c * NL:(c + 1) * NL, :].T)
        kc64[:, O_ZTSH:O_ZTSH + NL] = zsh
        kc64[:, O_ZTSL:O_ZTSL + NL] = zsl
        kc128 = k128c.copy()
        kc128[0:BL, O_CBT] = cbt[c * BL:(c + 1) * BL]
        in_maps.append(dict(pk64=kc64, pk128=kc128))
    return in_maps


_NC_CACHE = {}


def kernel(x, log_w, z, k, W1, b1, W2, b2, _trace=False, _trace_kwargs=None):
    assert int(k) == K
    in_maps = _host_prep(np.asarray(x, np.int32), np.asarray(z, np.float32),
                         np.asarray(W1, np.float32), np.asarray(b1, np.float32),
                         np.asarray(W2, np.float32), np.asarray(b2, np.float32))
    if "nc" not in _NC_CACHE:
        _NC_CACHE["nc"] = _build_nc()
    nc = _NC_CACHE["nc"]
    res = run_bass_kernel_spmd(
        nc, in_maps, list(range(P)), trace=_trace, **(_trace_kwargs or {}))
    if _trace:
        _NC_CACHE["last_result"] = res
    return np.concatenate([np.asarray(res.results[c]["out"], np.float32)
                           for c in range(P)])
